# revision 24
# baseline (speedup 1.0000x reference)
"""Distributed Trainium2 Bass kernel for a GIN message-passing layer.

Computation (see the GIN reference):
    agg     = segment_sum(x[src], dst, N)          # neighbor feature sums
    out     = relu((1 + eps) * x + agg)            # [N, D]
    pooled2 = segment_sum(out @ W + b, batch, B)   # [B, D]
returns (out, pooled2).

Distribution: `batch` is sorted, so each graph is a contiguous node range.
Core c owns graphs [c*GPC, (c+1)*GPC) and the corresponding node range;
edges are assigned to the core that owns their destination node, so the
aggregation is fully local.  x is replicated (bf16) so the random source
gathers are local.  Outputs are gathered on the host; no collectives.

Device algorithm (scatter-free):
  - The host reorders each core's local nodes into groups of 128 such
    that each group has at most `cap` in-edges per source-range bucket
    (bin packing; bucket A: src < half, bucket B: src >= half -- needed
    because dma_gather indices are int16).
  - Edges are laid out in a fixed-size slot stream per bucket:
    group g owns slots [g*cap, (g+1)*cap); empty slots hold src=0 and
    dst_rel=-1.
  - Device: dma_gather (bf16 rows, 1024-index calls round-robined over
    4 SWDGE queues -- a single queue serializes transfers at ~13ns/idx,
    4 queues reach ~3.3ns/idx; >1024 indices per call overflows the
    16KB descriptor ring and wedges the device).  For each gather tile a
    selection tensor sel[e, d] = (dst_rel[e] == d) is built with one
    vector compare against an iota matrix.  Per group, 2*cap/128 bf16
    matmuls sel_chunk^T @ x_chunk accumulate agg for its 128 nodes in
    f32 PSUM.  Then out = relu((1+eps)*x_f32 + agg) (scalar-engine relu)
    is written out and onehot^T @ out accumulates per-graph sums.
  - Epilogue: pooled2^T = W^T @ S^T + b outer cnt (cnt = per-graph node
    counts, host-provided index metadata).
"""

import math
from contextlib import ExitStack
from dataclasses import dataclass

import ml_dtypes
import numpy as np

import concourse.tile as tile
from concourse import bacc, mybir
from concourse.masks import make_identity

P = 128
F32 = mybir.dt.float32
BF16 = mybir.dt.bfloat16
I16 = mybir.dt.int16
ACT_COPY = mybir.ActivationFunctionType.Copy
ACT_RELU = mybir.ActivationFunctionType.Relu


@dataclass(frozen=True)
class Cfg:
    n_nodes: int
    n_graphs: int
    n_cores: int
    d: int
    half: int  # src index split so each gather view has < 32768 rows
    gtile: int  # gather slots per dma_gather call (multiple of 128, <= 1024)
    nq: int = 4  # SWDGE queues
    scratch: int = 16384  # SWDGE descriptor-ring carveout bytes (16B/desc)


REAL = Cfg(n_nodes=50000, n_graphs=64, n_cores=8, d=128, half=25000, gtile=1024)


# --------------------------------------------------------------------------
# host-side sharding
# --------------------------------------------------------------------------


def _wrap_idx(idx: np.ndarray) -> np.ndarray:
    """[n] -> [16, n/16] int16 layout for dma_gather: element i at [i%16, i//16]."""
    n = idx.shape[0]
    assert n % 16 == 0 and idx.min() >= -1 and idx.max() < 32768
    return np.ascontiguousarray(idx.reshape(n // 16, 16).T).astype(np.int16)


def _pack_groups(dA, dB, n_groups, cap):
    """Assign each node to a group: <=128 nodes/group, bucket loads <= cap.
    Returns group id per node, or None if infeasible."""
    n = dA.shape[0]
    grp = np.full(n, -1, dtype=np.int64)
    usedA = np.zeros(n_groups, dtype=np.int64)
    usedB = np.zeros(n_groups, dtype=np.int64)
    cnt = np.zeros(n_groups, dtype=np.int64)
    order = np.argsort(-(dA + dB), kind="stable")
    for i in order:
        ok = (cnt < P) & (usedA + dA[i] <= cap) & (usedB + dB[i] <= cap)
        if not ok.any():
            return None
        cand = np.where(ok)[0]
        j = cand[np.argmin(np.maximum(usedA[cand] + dA[i], usedB[cand] + dB[i]))]
        grp[i] = j
        usedA[j] += dA[i]
        usedB[j] += dB[i]
        cnt[j] += 1
    return grp


def _edge_slots(src, rel, egrp, n_groups, cap, pad_src):
    """Place edges (grouped by egrp) into the slot stream: group g owns
    slots [g*cap, (g+1)*cap).  Returns (src_slots, rel_slots)."""
    slots = n_groups * cap
    src_s = np.full(slots, pad_src, dtype=np.int64)
    rel_s = np.full(slots, -1.0, dtype=np.float32)
    order = np.argsort(egrp, kind="stable")
    eg = egrp[order]
    starts = np.searchsorted(eg, np.arange(n_groups))
    rank = np.arange(eg.shape[0]) - starts[eg]
    pos = eg * cap + rank
    src_s[pos] = src[order]
    rel_s[pos] = rel[order]
    return src_s, rel_s


def shard_inputs(cfg: Cfg, x, eps, W_pred, b_pred, edge_index, batch):
    gpc = cfg.n_graphs // cfg.n_cores
    d = cfg.d
    x = np.asarray(x, dtype=np.float32)
    xbf = x.astype(ml_dtypes.bfloat16)
    batch = np.asarray(batch).astype(np.int64)
    src = np.asarray(edge_index[0]).astype(np.int64)
    dst = np.asarray(edge_index[1]).astype(np.int64)

    assert (np.diff(batch) >= 0).all(), "batch must be sorted (graphs contiguous)"
    gstart = np.searchsorted(batch, np.arange(cfg.n_graphs + 1))
    core_start = gstart[np.arange(cfg.n_cores) * gpc]
    core_end = gstart[np.arange(cfg.n_cores) * gpc + gpc]
    counts = core_end - core_start
    n_loc_pad = max(P, int(math.ceil(counts.max() / P)) * P)
    n_groups = n_loc_pad // P

    ecore = batch[dst] // gpc
    per_core = []
    for c in range(cfg.n_cores):
        m = ecore == c
        s_c, dl_c = src[m], dst[m] - core_start[c]
        isA = s_c < cfg.half
        n = int(counts[c])
        dA = np.bincount(dl_c[isA], minlength=n)
        dB = np.bincount(dl_c[~isA], minlength=n)
        per_core.append((s_c, dl_c, isA, dA, dB))

    # shared bucket cap (multiple of 128) so all cores run one program
    cap = max(P, int(math.ceil(max(dA.max() for *_, dA, _ in per_core) / P)) * P,
              int(math.ceil(counts.max() * 16 / (2 * n_groups) / P)) * P)
    groups = None
    while groups is None:
        groups = []
        for c in range(cfg.n_cores):
            _, _, _, dA, dB = per_core[c]
            g = _pack_groups(dA, dB, n_groups, cap)
            if g is None:
                groups = None
                cap += P
                break
            groups.append(g)

    slots = n_groups * cap
    n_gt = int(math.ceil(slots / cfg.gtile))
    slots_pad = n_gt * cfg.gtile
    chunks_pad = slots_pad // P

    epsr = np.full((P, 1), np.asarray(eps).reshape(-1)[0], dtype=np.float32)
    brow = np.asarray(b_pred, dtype=np.float32).reshape(1, d)
    W = np.asarray(W_pred, dtype=np.float32)
    iota = np.tile(np.arange(P, dtype=np.float32), (P, 1))

    in_maps = []
    perms = []
    for c in range(cfg.n_cores):
        s_c, dl_c, isA, dA, dB = per_core[c]
        n = int(counts[c])
        grp = groups[c]
        order = np.argsort(grp, kind="stable")
        g_sorted = grp[order]
        starts = np.searchsorted(g_sorted, np.arange(n_groups))
        rank = np.arange(n) - starts[g_sorted]
        assert rank.max() < P
        new_id = g_sorted * P + rank
        perm = np.full(n_loc_pad, -1, dtype=np.int64)
        perm[new_id] = order  # perm[new] = old local id
        node_slot = np.empty(n, dtype=np.int64)
        node_slot[order] = rank

        rel = node_slot[dl_c].astype(np.float32)
        egrp = grp[dl_c]

        # dynamic-count mode (only when each call == one group-bucket):
        # trailing -1 slots are skipped on HW via a per-call valid count.
        # Counts are clamped to >=16 (each of the 16 DMA engines must get a
        # descriptor or the +16 completion semaphore never fires).
        dynamic = cap == cfg.gtile
        half = cfg.gtile // 2

        def bucket(mask, rebase):
            ss, rs = _edge_slots(s_c[mask] - rebase, rel[mask], egrp[mask],
                                 n_groups, cap, -1 if dynamic else 0)
            ss = np.concatenate([ss, np.zeros(slots_pad - slots, np.int64)])
            rs = np.concatenate([rs, np.full(slots_pad - slots, -1.0, np.float32)])
            cnts = []
            if dynamic:
                # per half-window valid counts; last window is emitted as two
                # half-size calls, others as one full call
                split_last = (cfg.gtile // P) % 2 == 0
                for w in range(n_groups):
                    s0 = w * cfg.gtile
                    parts = (
                        [(s0, half), (s0 + half, half)]
                        if (w == n_groups - 1 and split_last)
                        else [(s0, cfg.gtile)]
                    )
                    for ps, ln in parts:
                        cnt = int((ss[ps : ps + ln] >= 0).sum())
                        if cnt < 16:  # keep >=16 descriptors per call
                            ss[ps + cnt : ps + 16] = 0
                            cnt = 16
                        cnts.append(cnt)
            return np.tile(_wrap_idx(ss), (8, 1)), np.ascontiguousarray(
                rs.reshape(chunks_pad, P).T), cnts

        srcA, relA, cntsA = bucket(isA, 0)
        srcB, relB, cntsB = bucket(~isA, cfg.half)
        cnt_calls = None
        if dynamic:
            calls = []
            split_last = (cfg.gtile // P) % 2 == 0
            npc = [1] * (n_groups - 1) + [2 if split_last else 1]  # calls/window
            ia = ib = 0
            for g in range(n_groups):
                for _ in range(npc[g]):
                    calls.append(cntsA[ia]); ia += 1
                for _ in range(npc[g]):
                    calls.append(cntsB[ib]); ib += 1
            cnt_calls = np.asarray(calls, dtype=np.uint32).reshape(1, -1)

        xloc = np.zeros((n_loc_pad, d), dtype=np.float32)
        oneh = np.zeros((n_loc_pad, gpc), dtype=np.float32)
        valid = perm >= 0
        xloc[valid] = x[core_start[c] + perm[valid]]
        oneh[valid, batch[core_start[c] + perm[valid]] - c * gpc] = 1.0
        cnt_row = np.bincount(batch[core_start[c] : core_end[c]] - c * gpc,
                              minlength=gpc).astype(np.float32).reshape(1, gpc)

        im = {
                "x": xbf,
                "xloc": xloc,
                "oneh": oneh,
                "epsr": epsr,
                "W": W,
                "brow": brow,
                "cntrow": cnt_row,
                "iota": iota,
                "ccalls": cnt_calls,
                "srcA": srcA,
                "relA": relA,
                "srcB": srcB,
                "relB": relB,
            }
        if not dynamic:
            im.pop("ccalls")
        in_maps.append(im)
        perms.append(perm)

    meta = dict(
        core_start=core_start,
        counts=counts,
        n_loc_pad=n_loc_pad,
        cap=cap,
        slots_pad=slots_pad,
        perms=perms,
    )
    return in_maps, meta


def unshard(cfg: Cfg, results, meta):
    gpc = cfg.n_graphs // cfg.n_cores
    out = np.empty((cfg.n_nodes, cfg.d), dtype=np.float32)
    pooled2 = np.empty((cfg.n_graphs, cfg.d), dtype=np.float32)
    for c in range(cfg.n_cores):
        s = int(meta["core_start"][c])
        perm = meta["perms"][c]
        valid = perm >= 0
        out[s + perm[valid]] = results[c]["out_loc"][valid]
        pooled2[c * gpc : (c + 1) * gpc] = results[c]["pooledT"].T
    return out, pooled2


# --------------------------------------------------------------------------
# device graph
# --------------------------------------------------------------------------


def build_graph(cfg: Cfg, n_loc_pad: int, cap: int, slots_pad: int):
    gpc = cfg.n_graphs // cfg.n_cores
    d = cfg.d
    n_groups = n_loc_pad // P
    cpg = cap // P
    n_gt = slots_pad // cfg.gtile
    tchunks = cfg.gtile // P
    icols = slots_pad // 16
    chunks_pad = slots_pad // P

    nc = bacc.Bacc(
        "TRN2",
        target_bir_lowering=False,
        debug=False,
        enable_asserts=False,
        num_devices=cfg.n_cores,
        num_swdge_queues=cfg.nq,
        dynamic_dma_scratch_size=cfg.scratch,
    )

    x_ap = nc.dram_tensor("x", [cfg.n_nodes, d], BF16, kind="ExternalInput").ap()
    xloc_ap = nc.dram_tensor("xloc", [n_loc_pad, d], F32, kind="ExternalInput").ap()
    oneh_ap = nc.dram_tensor("oneh", [n_loc_pad, gpc], F32, kind="ExternalInput").ap()
    epsr_ap = nc.dram_tensor("epsr", [P, 1], F32, kind="ExternalInput").ap()
    W_ap = nc.dram_tensor("W", [d, d], F32, kind="ExternalInput").ap()
    brow_ap = nc.dram_tensor("brow", [1, d], F32, kind="ExternalInput").ap()
    cnt_ap = nc.dram_tensor("cntrow", [1, gpc], F32, kind="ExternalInput").ap()
    iota_ap = nc.dram_tensor("iota", [P, P], F32, kind="ExternalInput").ap()
    idx_aps = {
        name: nc.dram_tensor(name, [P, icols], I16, kind="ExternalInput").ap()
        for name in ("srcA", "srcB")
    }
    rel_aps = {
        name: nc.dram_tensor(name, [P, chunks_pad], F32, kind="ExternalInput").ap()
        for name in ("relA", "relB")
    }
    dynamic = cap == cfg.gtile
    n_calls = 2 * (n_gt + (1 if (cfg.gtile // P) % 2 == 0 else 0))
    ccalls_ap = (
        nc.dram_tensor("ccalls", [1, n_calls], mybir.dt.uint32,
                       kind="ExternalInput").ap()
        if dynamic else None
    )

    out_ap = nc.dram_tensor("out_loc", [n_loc_pad, d], F32, kind="ExternalOutput").ap()
    pooledT_ap = nc.dram_tensor("pooledT", [d, gpc], F32, kind="ExternalOutput").ap()

    xviews = {"A": x_ap[0 : cfg.half], "B": x_ap[cfg.half : cfg.n_nodes]}

    with tile.TileContext(nc) as tc, ExitStack() as ctx:
        const_pool = ctx.enter_context(tc.tile_pool(name="const", bufs=1))
        idx_pool = ctx.enter_context(tc.tile_pool(name="idx", bufs=1))
        gpools = {
            b: ctx.enter_context(tc.tile_pool(name=f"g{b}", bufs=2 * cfg.nq))
            for b in "AB"
        }
        spools = {
            b: ctx.enter_context(tc.tile_pool(name=f"s{b}", bufs=2 * cfg.nq))
            for b in "AB"
        }
        node_pool = ctx.enter_context(tc.tile_pool(name="node", bufs=3))
        small = ctx.enter_context(tc.tile_pool(name="small", bufs=1))
        psum_pool = ctx.enter_context(tc.tile_pool(name="psum", bufs=4, space="PSUM"))
        psum_ep = ctx.enter_context(tc.tile_pool(name="psum_ep", bufs=1, space="PSUM"))
        psum_s = ctx.enter_context(tc.tile_pool(name="psum_s", bufs=1, space="PSUM"))

        # ---- edge indices first (gathers depend on them); scalar-engine
        # HWDGE so they are not queued behind other input DMAs.  The first
        # FIRSTW windows live in a separate small tile so early gathers only
        # wait for a 128KB DMA, not the whole index stream.
        wcols = cfg.gtile // 16
        firstw = min(8, n_gt)
        idx_sb = {}
        for b in "AB":
            t1 = idx_pool.tile([P, firstw * wcols], I16, tag=f"src{b}1")
            nc.scalar.dma_start(t1[:], idx_aps[f"src{b}"][:, 0 : firstw * wcols])
            if icols > firstw * wcols:
                t2 = idx_pool.tile([P, icols - firstw * wcols], I16, tag=f"src{b}2")
                nc.scalar.dma_start(t2[:], idx_aps[f"src{b}"][:, firstw * wcols :])
            else:
                t2 = None
            idx_sb[b] = (t1, t2)
        rel_sb = {}
        for b in "AB":
            t = idx_pool.tile([P, chunks_pad], F32, tag=f"rel{b}")
            nc.scalar.dma_start(t[:], rel_aps[f"rel{b}"][:, :])
            rel_sb[b] = t
        if dynamic:
            ccalls_sb = idx_pool.tile([1, n_calls], mybir.dt.uint32, tag="cc")
            nc.scalar.dma_start(ccalls_sb[:], ccalls_ap[:, :])
            cnt_reg = list(nc.alloc_registers("nidx_dyn",
                                              [mybir.EngineType.Pool]))[0]

        # ---- constants
        ep = const_pool.tile([P, 1], F32)
        nc.sync.dma_start(ep[:], epsr_ap[:, :])
        eps1 = const_pool.tile([P, 1], F32)
        nc.vector.tensor_scalar_add(eps1[:], ep[:], 1.0)
        iota_sb = const_pool.tile([P, P], F32)
        nc.sync.dma_start(iota_sb[:], iota_ap[:, :])

        # ---- lazy gather + sel tiles per bucket; queues round-robin.
        # Tile assigns DMASW sem lanes to Pool-engine DMAs round-robin in
        # SCHEDULED order; chain gathers in emission order so lane k%8 always
        # pairs with queue k%nq (a sem lane is locked to one SWDGE queue).
        window: dict = {}
        qcounter = [0]
        last_gather = [None]
        nidx_reg = nc.gpsimd.to_reg(cfg.gtile)
        nidx_half = nc.gpsimd.to_reg(cfg.gtile // 2)

        def chain(gi, bump=True):
            if last_gather[0] is not None:
                tile.add_dep_helper(
                    gi.ins, last_gather[0].ins, sync=False,
                    reason="swdge lane/queue pairing: keep emission order",
                )
            last_gather[0] = gi
            if bump:
                qcounter[0] += 1

        def count_reg():
            if not dynamic:
                return None
            ci = qcounter[0]
            li = nc.gpsimd.reg_load(cnt_reg, ccalls_sb[0:1, ci : ci + 1])
            chain(li, bump=False)
            return cnt_reg

        first_use = {"A": 0, "B": 0}

        def get_window(b: str, ti: int):
            key = (b, ti)
            if key not in window:
                g = gpools[b].tile([P, tchunks, d], BF16, tag=f"g{b}")
                if dynamic and first_use[b] < 2 * cfg.nq:
                    # skipped trailing slots are never written by the gather;
                    # zero each pool buffer once so the zero-weighted matmul
                    # never multiplies uninitialized (possibly NaN) data
                    nc.vector.memset(g[:], 0)
                    first_use[b] += 1
                if ti < firstw:
                    isb = idx_sb[b][0][:, ti * wcols : (ti + 1) * wcols]
                else:
                    tj = ti - firstw
                    isb = idx_sb[b][1][:, tj * wcols : (tj + 1) * wcols]
                if ti == n_gt - 1 and tchunks % 2 == 0:
                    # last window of the bucket: split across two queues so the
                    # stream's tail drains in parallel instead of serializing
                    # ~gtile descriptors on a single queue
                    h = tchunks // 2
                    r = count_reg()
                    chain(nc.gpsimd.dma_gather(
                        g[:, 0:h, :], xviews[b], isb[:, : wcols // 2],
                        cfg.gtile // 2, r if dynamic else nidx_half, d,
                        queue_num=qcounter[0] % cfg.nq,
                    ))
                    r = count_reg()
                    chain(nc.gpsimd.dma_gather(
                        g[:, h:tchunks, :], xviews[b], isb[:, wcols // 2 :],
                        cfg.gtile // 2, r if dynamic else nidx_half, d,
                        queue_num=qcounter[0] % cfg.nq,
                    ))
                else:
                    r = count_reg()
                    chain(nc.gpsimd.dma_gather(
                        g[:], xviews[b], isb, cfg.gtile, r if dynamic else nidx_reg, d,
                        queue_num=qcounter[0] % cfg.nq,
                    ))
                sel = spools[b].tile([P, tchunks, d], BF16, tag=f"s{b}")
                ch = slice(ti * tchunks, (ti + 1) * tchunks)
                nc.vector.tensor_tensor(
                    out=sel[:],
                    in0=rel_sb[b][:, ch, None].to_broadcast([P, tchunks, d]),
                    in1=iota_sb[:][:, None, :].to_broadcast([P, tchunks, d]),
                    op=mybir.AluOpType.is_equal,
                )
                window[key] = (g, sel)
            return window[key]

        # ---- per-group aggregation + node update
        S_psum = psum_s.tile([gpc, d], F32)
        for grp in range(n_groups):
            agg = psum_pool.tile([P, d], F32, tag="agg")
            k = 0
            for b in "AB":
                for cchunk in range(grp * cpg, (grp + 1) * cpg):
                    ti, sl = divmod(cchunk, tchunks)
                    g, sel = get_window(b, ti)
                    nc.tensor.matmul(
                        agg[:],
                        lhsT=sel[:, sl, :],
                        rhs=g[:, sl, :],
                        start=(k == 0),
                        stop=(k == 2 * cpg - 1),
                    )
                    k += 1

            rsl = slice(grp * P, (grp + 1) * P)
            xt = node_pool.tile([P, d], F32, tag="xt")
            nc.sync.dma_start(xt[:], xloc_ap[rsl, :])
            oh = node_pool.tile([P, gpc], F32, tag="oh")
            nc.sync.dma_start(oh[:], oneh_ap[rsl, :])
            xs = node_pool.tile([P, d], F32, tag="xs")
            nc.scalar.activation(xs[:], xt[:], ACT_COPY, scale=eps1[:])
            tsum = node_pool.tile([P, d], F32, tag="tsum")
            nc.vector.tensor_add(tsum[:], xs[:], agg[:])
            ot = node_pool.tile([P, d], F32, tag="ot")
            nc.scalar.activation(ot[:], tsum[:], ACT_RELU)
            nc.sync.dma_start(out_ap[rsl, :], ot[:])
            nc.tensor.matmul(
                S_psum[:],
                lhsT=oh[:],
                rhs=ot[:],
                start=(grp == 0),
                stop=(grp == n_groups - 1),
            )

        # ---- pooling epilogue: pooled2^T = W^T @ S^T + b outer cnt
        Wsb = const_pool.tile([P, d], F32)
        nc.sync.dma_start(Wsb[:], W_ap[:, :])
        brow = const_pool.tile([1, d], F32)
        nc.sync.dma_start(brow[:], brow_ap[:, :])
        cnt_row = const_pool.tile([1, gpc], F32)
        nc.sync.dma_start(cnt_row[:], cnt_ap[:, :])
        ident = const_pool.tile([P, P], F32)
        make_identity(nc, ident[:])
        S_sb = small.tile([gpc, d], F32)
        nc.vector.tensor_copy(S_sb[:], S_psum[:])
        ST_ps = psum_ep.tile([P, gpc], F32, tag="T1")
        nc.tensor.transpose(ST_ps[:], S_sb[:, 0:d], ident[0:gpc, 0:gpc])
        ST_sb = small.tile([P, gpc], F32)
        nc.vector.tensor_copy(ST_sb[:], ST_ps[:])
        P_ps = psum_ep.tile([P, gpc], F32, tag="T3")
        nc.tensor.matmul(P_ps[:], lhsT=Wsb[:], rhs=ST_sb[:], start=True, stop=False)
        nc.tensor.matmul(P_ps[:], lhsT=brow[:], rhs=cnt_row[:], start=False, stop=True)
        P_sb = small.tile([P, gpc], F32)
        nc.vector.tensor_copy(P_sb[:], P_ps[:])
        nc.sync.dma_start(pooledT_ap[:, :], P_sb[:])

    nc.compile()
    return nc


# --------------------------------------------------------------------------
# entry point
# --------------------------------------------------------------------------

_graph_cache: dict = {}


def _get_graph(cfg: Cfg, n_loc_pad: int, cap: int, slots_pad: int):
    key = (cfg, n_loc_pad, cap, slots_pad)
    if key not in _graph_cache:
        _graph_cache[key] = build_graph(cfg, n_loc_pad, cap, slots_pad)
    return _graph_cache[key]


def kernel(x, eps, W_pred, b_pred, edge_index, batch):
    from concourse import bass_utils

    cfg = REAL
    in_maps, meta = shard_inputs(cfg, x, eps, W_pred, b_pred, edge_index, batch)
    nc = _get_graph(cfg, meta["n_loc_pad"], meta["cap"], meta["slots_pad"])
    res = bass_utils.run_bass_kernel_spmd(
        nc, in_maps, core_ids=list(range(cfg.n_cores))
    )
    return unshard(cfg, res.results, meta)


# revision 25
# speedup vs baseline: 1.3189x; 1.3189x over previous
"""Distributed Trainium2 Bass kernel for a GIN message-passing layer.

Computation (see the GIN reference):
    agg     = segment_sum(x[src], dst, N)          # neighbor feature sums
    out     = relu((1 + eps) * x + agg)            # [N, D]
    pooled2 = segment_sum(out @ W + b, batch, B)   # [B, D]
returns (out, pooled2).

Distribution: `batch` is sorted, so each graph is a contiguous node range.
Core c owns graphs [c*GPC, (c+1)*GPC) and the corresponding node range;
edges are assigned to the core that owns their destination node, so the
aggregation is fully local.  x is replicated (bf16) so the random source
gathers are local.  Outputs are gathered on the host; no collectives.

Device algorithm (scatter-free):
  - The host reorders each core's local nodes into groups of 128 such
    that each group has at most `cap` in-edges per source-range bucket
    (bin packing; bucket A: src < half, bucket B: src >= half -- needed
    because dma_gather indices are int16).
  - Edges are laid out in a fixed-size slot stream per bucket:
    group g owns slots [g*cap, (g+1)*cap); empty slots hold src=0 and
    dst_rel=-1.
  - Device: dma_gather (bf16 rows, 1024-index calls round-robined over
    4 SWDGE queues -- a single queue serializes transfers at ~13ns/idx,
    4 queues reach ~3.3ns/idx; >1024 indices per call overflows the
    16KB descriptor ring and wedges the device).  For each gather tile a
    selection tensor sel[e, d] = (dst_rel[e] == d) is built with one
    vector compare against an iota matrix.  Per group, 2*cap/128 bf16
    matmuls sel_chunk^T @ x_chunk accumulate agg for its 128 nodes in
    f32 PSUM.  Then out = relu((1+eps)*x_f32 + agg) (scalar-engine relu)
    is written out and onehot^T @ out accumulates per-graph sums.
  - Epilogue: pooled2^T = W^T @ S^T + b outer cnt (cnt = per-graph node
    counts, host-provided index metadata).
"""

import math
from contextlib import ExitStack
from dataclasses import dataclass

import ml_dtypes
import numpy as np

import concourse.tile as tile
from concourse import bacc, mybir
from concourse.masks import make_identity

P = 128
F32 = mybir.dt.float32
BF16 = mybir.dt.bfloat16
I16 = mybir.dt.int16
ACT_COPY = mybir.ActivationFunctionType.Copy
ACT_RELU = mybir.ActivationFunctionType.Relu


@dataclass(frozen=True)
class Cfg:
    n_nodes: int
    n_graphs: int
    n_cores: int
    d: int
    half: int  # src index split so each gather view has < 32768 rows
    gtile: int  # gather slots per dma_gather call (multiple of 128, <= 1024)
    nq: int = 4  # SWDGE queues
    scratch: int = 16384  # SWDGE descriptor-ring carveout bytes (16B/desc)


REAL = Cfg(n_nodes=50000, n_graphs=64, n_cores=8, d=128, half=25000, gtile=1024)


# --------------------------------------------------------------------------
# host-side sharding
# --------------------------------------------------------------------------


def _wrap_idx(idx: np.ndarray) -> np.ndarray:
    """[n] -> [16, n/16] int16 layout for dma_gather: element i at [i%16, i//16]."""
    n = idx.shape[0]
    assert n % 16 == 0 and idx.min() >= -1 and idx.max() < 32768
    return np.ascontiguousarray(idx.reshape(n // 16, 16).T).astype(np.int16)


def _pack_groups(dA, dB, n_groups, cap):
    """Assign each node to a group: <=128 nodes/group, bucket loads <= cap.
    Returns group id per node, or None if infeasible."""
    n = dA.shape[0]
    grp = np.full(n, -1, dtype=np.int64)
    usedA = np.zeros(n_groups, dtype=np.int64)
    usedB = np.zeros(n_groups, dtype=np.int64)
    cnt = np.zeros(n_groups, dtype=np.int64)
    order = np.argsort(-(dA + dB), kind="stable")
    for i in order:
        ok = (cnt < P) & (usedA + dA[i] <= cap) & (usedB + dB[i] <= cap)
        if not ok.any():
            return None
        cand = np.where(ok)[0]
        j = cand[np.argmin(np.maximum(usedA[cand] + dA[i], usedB[cand] + dB[i]))]
        grp[i] = j
        usedA[j] += dA[i]
        usedB[j] += dB[i]
        cnt[j] += 1
    return grp


def _edge_slots(src, rel, egrp, n_groups, cap, pad_src):
    """Place edges (grouped by egrp) into the slot stream: group g owns
    slots [g*cap, (g+1)*cap).  Returns (src_slots, rel_slots)."""
    slots = n_groups * cap
    src_s = np.full(slots, pad_src, dtype=np.int64)
    rel_s = np.full(slots, -1.0, dtype=np.float32)
    order = np.argsort(egrp, kind="stable")
    eg = egrp[order]
    starts = np.searchsorted(eg, np.arange(n_groups))
    rank = np.arange(eg.shape[0]) - starts[eg]
    pos = eg * cap + rank
    src_s[pos] = src[order]
    rel_s[pos] = rel[order]
    return src_s, rel_s


def shard_inputs(cfg: Cfg, x, eps, W_pred, b_pred, edge_index, batch):
    gpc = cfg.n_graphs // cfg.n_cores
    d = cfg.d
    x = np.asarray(x, dtype=np.float32)
    xbf = x.astype(ml_dtypes.bfloat16)
    batch = np.asarray(batch).astype(np.int64)
    src = np.asarray(edge_index[0]).astype(np.int64)
    dst = np.asarray(edge_index[1]).astype(np.int64)

    assert (np.diff(batch) >= 0).all(), "batch must be sorted (graphs contiguous)"
    gstart = np.searchsorted(batch, np.arange(cfg.n_graphs + 1))
    core_start = gstart[np.arange(cfg.n_cores) * gpc]
    core_end = gstart[np.arange(cfg.n_cores) * gpc + gpc]
    counts = core_end - core_start
    n_loc_pad = max(P, int(math.ceil(counts.max() / P)) * P)
    n_groups = n_loc_pad // P

    ecore = batch[dst] // gpc
    per_core = []
    for c in range(cfg.n_cores):
        m = ecore == c
        s_c, dl_c = src[m], dst[m] - core_start[c]
        isA = s_c < cfg.half
        n = int(counts[c])
        dA = np.bincount(dl_c[isA], minlength=n)
        dB = np.bincount(dl_c[~isA], minlength=n)
        per_core.append((s_c, dl_c, isA, dA, dB))

    # shared bucket cap (multiple of 128) so all cores run one program
    cap = max(P, int(math.ceil(max(dA.max() for *_, dA, _ in per_core) / P)) * P,
              int(math.ceil(counts.max() * 16 / (2 * n_groups) / P)) * P)
    groups = None
    while groups is None:
        groups = []
        for c in range(cfg.n_cores):
            _, _, _, dA, dB = per_core[c]
            g = _pack_groups(dA, dB, n_groups, cap)
            if g is None:
                groups = None
                cap += P
                break
            groups.append(g)

    slots = n_groups * cap
    n_gt = int(math.ceil(slots / cfg.gtile))
    slots_pad = n_gt * cfg.gtile
    chunks_pad = slots_pad // P

    epsr = np.full((P, 1), np.asarray(eps).reshape(-1)[0], dtype=np.float32)
    brow = np.asarray(b_pred, dtype=np.float32).reshape(1, d)
    W = np.asarray(W_pred, dtype=np.float32)
    iota = np.tile(np.arange(P, dtype=np.float32), (P, 1))

    in_maps = []
    perms = []
    for c in range(cfg.n_cores):
        s_c, dl_c, isA, dA, dB = per_core[c]
        n = int(counts[c])
        grp = groups[c]
        order = np.argsort(grp, kind="stable")
        g_sorted = grp[order]
        starts = np.searchsorted(g_sorted, np.arange(n_groups))
        rank = np.arange(n) - starts[g_sorted]
        assert rank.max() < P
        new_id = g_sorted * P + rank
        perm = np.full(n_loc_pad, -1, dtype=np.int64)
        perm[new_id] = order  # perm[new] = old local id
        node_slot = np.empty(n, dtype=np.int64)
        node_slot[order] = rank

        rel = node_slot[dl_c].astype(np.float32)
        egrp = grp[dl_c]

        # dynamic-count mode (only when each call == one group-bucket):
        # trailing -1 slots are skipped on HW via a per-call valid count.
        # Counts are clamped to >=16 (each of the 16 DMA engines must get a
        # descriptor or the +16 completion semaphore never fires).
        # dynamic per-call counts measured SLOWER on HW (354us vs 272us):
        # the per-call gpsimd reg_load outweighs the 4% descriptor savings
        dynamic = False
        half = cfg.gtile // 2

        def bucket(mask, rebase):
            ss, rs = _edge_slots(s_c[mask] - rebase, rel[mask], egrp[mask],
                                 n_groups, cap, -1 if dynamic else 0)
            ss = np.concatenate([ss, np.zeros(slots_pad - slots, np.int64)])
            rs = np.concatenate([rs, np.full(slots_pad - slots, -1.0, np.float32)])
            cnts = []
            if dynamic:
                # per half-window valid counts; last window is emitted as two
                # half-size calls, others as one full call
                split_last = (cfg.gtile // P) % 2 == 0
                for w in range(n_groups):
                    s0 = w * cfg.gtile
                    parts = (
                        [(s0, half), (s0 + half, half)]
                        if (w == n_groups - 1 and split_last)
                        else [(s0, cfg.gtile)]
                    )
                    for ps, ln in parts:
                        cnt = int((ss[ps : ps + ln] >= 0).sum())
                        if cnt < 16:  # keep >=16 descriptors per call
                            ss[ps + cnt : ps + 16] = 0
                            cnt = 16
                        cnts.append(cnt)
            return np.tile(_wrap_idx(ss), (8, 1)), np.ascontiguousarray(
                rs.reshape(chunks_pad, P).T), cnts

        srcA, relA, cntsA = bucket(isA, 0)
        srcB, relB, cntsB = bucket(~isA, cfg.half)
        cnt_calls = None
        if dynamic:
            calls = []
            split_last = (cfg.gtile // P) % 2 == 0
            npc = [1] * (n_groups - 1) + [2 if split_last else 1]  # calls/window
            ia = ib = 0
            for g in range(n_groups):
                for _ in range(npc[g]):
                    calls.append(cntsA[ia]); ia += 1
                for _ in range(npc[g]):
                    calls.append(cntsB[ib]); ib += 1
            cnt_calls = np.asarray(calls, dtype=np.uint32).reshape(1, -1)

        xloc = np.zeros((n_loc_pad, d), dtype=np.float32)
        oneh = np.zeros((n_loc_pad, gpc), dtype=np.float32)
        valid = perm >= 0
        xloc[valid] = x[core_start[c] + perm[valid]]
        oneh[valid, batch[core_start[c] + perm[valid]] - c * gpc] = 1.0
        cnt_row = np.bincount(batch[core_start[c] : core_end[c]] - c * gpc,
                              minlength=gpc).astype(np.float32).reshape(1, gpc)

        im = {
                "x": xbf,
                "xloc": xloc,
                "oneh": oneh,
                "epsr": epsr,
                "W": W,
                "brow": brow,
                "cntrow": cnt_row,
                "iota": iota,
                "ccalls": cnt_calls,
                "srcA": srcA,
                "relA": relA,
                "srcB": srcB,
                "relB": relB,
            }
        if not dynamic:
            im.pop("ccalls")
        in_maps.append(im)
        perms.append(perm)

    meta = dict(
        core_start=core_start,
        counts=counts,
        n_loc_pad=n_loc_pad,
        cap=cap,
        slots_pad=slots_pad,
        perms=perms,
    )
    return in_maps, meta


def unshard(cfg: Cfg, results, meta):
    gpc = cfg.n_graphs // cfg.n_cores
    out = np.empty((cfg.n_nodes, cfg.d), dtype=np.float32)
    pooled2 = np.empty((cfg.n_graphs, cfg.d), dtype=np.float32)
    for c in range(cfg.n_cores):
        s = int(meta["core_start"][c])
        perm = meta["perms"][c]
        valid = perm >= 0
        out[s + perm[valid]] = results[c]["out_loc"][valid]
        pooled2[c * gpc : (c + 1) * gpc] = results[c]["pooledT"].T
    return out, pooled2


# --------------------------------------------------------------------------
# device graph
# --------------------------------------------------------------------------


def build_graph(cfg: Cfg, n_loc_pad: int, cap: int, slots_pad: int):
    gpc = cfg.n_graphs // cfg.n_cores
    d = cfg.d
    n_groups = n_loc_pad // P
    cpg = cap // P
    n_gt = slots_pad // cfg.gtile
    tchunks = cfg.gtile // P
    icols = slots_pad // 16
    chunks_pad = slots_pad // P

    nc = bacc.Bacc(
        "TRN2",
        target_bir_lowering=False,
        debug=False,
        enable_asserts=False,
        num_devices=cfg.n_cores,
        num_swdge_queues=cfg.nq,
        dynamic_dma_scratch_size=cfg.scratch,
    )

    x_ap = nc.dram_tensor("x", [cfg.n_nodes, d], BF16, kind="ExternalInput").ap()
    xloc_ap = nc.dram_tensor("xloc", [n_loc_pad, d], F32, kind="ExternalInput").ap()
    oneh_ap = nc.dram_tensor("oneh", [n_loc_pad, gpc], F32, kind="ExternalInput").ap()
    epsr_ap = nc.dram_tensor("epsr", [P, 1], F32, kind="ExternalInput").ap()
    W_ap = nc.dram_tensor("W", [d, d], F32, kind="ExternalInput").ap()
    brow_ap = nc.dram_tensor("brow", [1, d], F32, kind="ExternalInput").ap()
    cnt_ap = nc.dram_tensor("cntrow", [1, gpc], F32, kind="ExternalInput").ap()
    iota_ap = nc.dram_tensor("iota", [P, P], F32, kind="ExternalInput").ap()
    idx_aps = {
        name: nc.dram_tensor(name, [P, icols], I16, kind="ExternalInput").ap()
        for name in ("srcA", "srcB")
    }
    rel_aps = {
        name: nc.dram_tensor(name, [P, chunks_pad], F32, kind="ExternalInput").ap()
        for name in ("relA", "relB")
    }
    dynamic = False  # see shard_inputs: reg_load cost > descriptor savings
    n_calls = 2 * (n_gt + (1 if (cfg.gtile // P) % 2 == 0 else 0))
    ccalls_ap = (
        nc.dram_tensor("ccalls", [1, n_calls], mybir.dt.uint32,
                       kind="ExternalInput").ap()
        if dynamic else None
    )

    out_ap = nc.dram_tensor("out_loc", [n_loc_pad, d], F32, kind="ExternalOutput").ap()
    pooledT_ap = nc.dram_tensor("pooledT", [d, gpc], F32, kind="ExternalOutput").ap()

    xviews = {"A": x_ap[0 : cfg.half], "B": x_ap[cfg.half : cfg.n_nodes]}

    with tile.TileContext(nc) as tc, ExitStack() as ctx:
        const_pool = ctx.enter_context(tc.tile_pool(name="const", bufs=1))
        idx_pool = ctx.enter_context(tc.tile_pool(name="idx", bufs=1))
        gpools = {
            b: ctx.enter_context(tc.tile_pool(name=f"g{b}", bufs=2 * cfg.nq))
            for b in "AB"
        }
        spools = {
            b: ctx.enter_context(tc.tile_pool(name=f"s{b}", bufs=2 * cfg.nq))
            for b in "AB"
        }
        node_pool = ctx.enter_context(tc.tile_pool(name="node", bufs=3))
        small = ctx.enter_context(tc.tile_pool(name="small", bufs=1))
        psum_pool = ctx.enter_context(tc.tile_pool(name="psum", bufs=4, space="PSUM"))
        psum_ep = ctx.enter_context(tc.tile_pool(name="psum_ep", bufs=1, space="PSUM"))
        psum_s = ctx.enter_context(tc.tile_pool(name="psum_s", bufs=1, space="PSUM"))

        # ---- edge indices first (gathers depend on them); scalar-engine
        # HWDGE so they are not queued behind other input DMAs.  The first
        # FIRSTW windows live in a separate small tile so early gathers only
        # wait for a 128KB DMA, not the whole index stream.
        wcols = cfg.gtile // 16
        firstw = min(8, n_gt)
        idx_sb = {}
        for b in "AB":
            t1 = idx_pool.tile([P, firstw * wcols], I16, tag=f"src{b}1")
            nc.scalar.dma_start(t1[:], idx_aps[f"src{b}"][:, 0 : firstw * wcols])
            if icols > firstw * wcols:
                t2 = idx_pool.tile([P, icols - firstw * wcols], I16, tag=f"src{b}2")
                nc.scalar.dma_start(t2[:], idx_aps[f"src{b}"][:, firstw * wcols :])
            else:
                t2 = None
            idx_sb[b] = (t1, t2)
        rel_sb = {}
        for b in "AB":
            t = idx_pool.tile([P, chunks_pad], F32, tag=f"rel{b}")
            nc.scalar.dma_start(t[:], rel_aps[f"rel{b}"][:, :])
            rel_sb[b] = t
        if dynamic:
            ccalls_sb = idx_pool.tile([1, n_calls], mybir.dt.uint32, tag="cc")
            nc.scalar.dma_start(ccalls_sb[:], ccalls_ap[:, :])
            cnt_reg = list(nc.alloc_registers("nidx_dyn",
                                              [mybir.EngineType.Pool]))[0]

        # ---- constants
        ep = const_pool.tile([P, 1], F32)
        nc.sync.dma_start(ep[:], epsr_ap[:, :])
        eps1 = const_pool.tile([P, 1], F32)
        nc.vector.tensor_scalar_add(eps1[:], ep[:], 1.0)
        iota_sb = const_pool.tile([P, P], F32)
        nc.sync.dma_start(iota_sb[:], iota_ap[:, :])

        # ---- lazy gather + sel tiles per bucket; queues round-robin.
        # Tile assigns DMASW sem lanes to Pool-engine DMAs round-robin in
        # SCHEDULED order; chain gathers in emission order so lane k%8 always
        # pairs with queue k%nq (a sem lane is locked to one SWDGE queue).
        window: dict = {}
        qcounter = [0]
        last_gather = [None]
        nidx_reg = nc.gpsimd.to_reg(cfg.gtile)
        nidx_half = nc.gpsimd.to_reg(cfg.gtile // 2)

        def chain(gi, bump=True):
            if last_gather[0] is not None:
                tile.add_dep_helper(
                    gi.ins, last_gather[0].ins, sync=False,
                    reason="swdge lane/queue pairing: keep emission order",
                )
            last_gather[0] = gi
            if bump:
                qcounter[0] += 1

        def count_reg():
            if not dynamic:
                return None
            ci = qcounter[0]
            li = nc.gpsimd.reg_load(cnt_reg, ccalls_sb[0:1, ci : ci + 1])
            chain(li, bump=False)
            return cnt_reg

        first_use = {"A": 0, "B": 0}

        def get_window(b: str, ti: int):
            key = (b, ti)
            if key not in window:
                g = gpools[b].tile([P, tchunks, d], BF16, tag=f"g{b}")
                if dynamic and first_use[b] < 2 * cfg.nq:
                    # skipped trailing slots are never written by the gather;
                    # zero each pool buffer once so the zero-weighted matmul
                    # never multiplies uninitialized (possibly NaN) data
                    nc.vector.memset(g[:], 0)
                    first_use[b] += 1
                if ti < firstw:
                    isb = idx_sb[b][0][:, ti * wcols : (ti + 1) * wcols]
                else:
                    tj = ti - firstw
                    isb = idx_sb[b][1][:, tj * wcols : (tj + 1) * wcols]
                if ti == n_gt - 1 and tchunks % 2 == 0:
                    # last window of the bucket: split across two queues so the
                    # stream's tail drains in parallel instead of serializing
                    # ~gtile descriptors on a single queue
                    h = tchunks // 2
                    r = count_reg()
                    chain(nc.gpsimd.dma_gather(
                        g[:, 0:h, :], xviews[b], isb[:, : wcols // 2],
                        cfg.gtile // 2, r if dynamic else nidx_half, d,
                        queue_num=qcounter[0] % cfg.nq,
                    ))
                    r = count_reg()
                    chain(nc.gpsimd.dma_gather(
                        g[:, h:tchunks, :], xviews[b], isb[:, wcols // 2 :],
                        cfg.gtile // 2, r if dynamic else nidx_half, d,
                        queue_num=qcounter[0] % cfg.nq,
                    ))
                else:
                    r = count_reg()
                    chain(nc.gpsimd.dma_gather(
                        g[:], xviews[b], isb, cfg.gtile, r if dynamic else nidx_reg, d,
                        queue_num=qcounter[0] % cfg.nq,
                    ))
                sel = spools[b].tile([P, tchunks, d], BF16, tag=f"s{b}")
                ch = slice(ti * tchunks, (ti + 1) * tchunks)
                nc.vector.tensor_tensor(
                    out=sel[:],
                    in0=rel_sb[b][:, ch, None].to_broadcast([P, tchunks, d]),
                    in1=iota_sb[:][:, None, :].to_broadcast([P, tchunks, d]),
                    op=mybir.AluOpType.is_equal,
                )
                window[key] = (g, sel)
            return window[key]

        # ---- per-group aggregation + node update
        S_psum = psum_s.tile([gpc, d], F32)
        for grp in range(n_groups):
            agg = psum_pool.tile([P, d], F32, tag="agg")
            k = 0
            for b in "AB":
                for cchunk in range(grp * cpg, (grp + 1) * cpg):
                    ti, sl = divmod(cchunk, tchunks)
                    g, sel = get_window(b, ti)
                    nc.tensor.matmul(
                        agg[:],
                        lhsT=sel[:, sl, :],
                        rhs=g[:, sl, :],
                        start=(k == 0),
                        stop=(k == 2 * cpg - 1),
                    )
                    k += 1

            rsl = slice(grp * P, (grp + 1) * P)
            xt = node_pool.tile([P, d], F32, tag="xt")
            nc.sync.dma_start(xt[:], xloc_ap[rsl, :])
            oh = node_pool.tile([P, gpc], F32, tag="oh")
            nc.sync.dma_start(oh[:], oneh_ap[rsl, :])
            xs = node_pool.tile([P, d], F32, tag="xs")
            nc.scalar.activation(xs[:], xt[:], ACT_COPY, scale=eps1[:])
            tsum = node_pool.tile([P, d], F32, tag="tsum")
            nc.vector.tensor_add(tsum[:], xs[:], agg[:])
            ot = node_pool.tile([P, d], F32, tag="ot")
            nc.scalar.activation(ot[:], tsum[:], ACT_RELU)
            nc.sync.dma_start(out_ap[rsl, :], ot[:])
            nc.tensor.matmul(
                S_psum[:],
                lhsT=oh[:],
                rhs=ot[:],
                start=(grp == 0),
                stop=(grp == n_groups - 1),
            )

        # ---- pooling epilogue: pooled2^T = W^T @ S^T + b outer cnt
        Wsb = const_pool.tile([P, d], F32)
        nc.sync.dma_start(Wsb[:], W_ap[:, :])
        brow = const_pool.tile([1, d], F32)
        nc.sync.dma_start(brow[:], brow_ap[:, :])
        cnt_row = const_pool.tile([1, gpc], F32)
        nc.sync.dma_start(cnt_row[:], cnt_ap[:, :])
        ident = const_pool.tile([P, P], F32)
        make_identity(nc, ident[:])
        S_sb = small.tile([gpc, d], F32)
        nc.vector.tensor_copy(S_sb[:], S_psum[:])
        ST_ps = psum_ep.tile([P, gpc], F32, tag="T1")
        nc.tensor.transpose(ST_ps[:], S_sb[:, 0:d], ident[0:gpc, 0:gpc])
        ST_sb = small.tile([P, gpc], F32)
        nc.vector.tensor_copy(ST_sb[:], ST_ps[:])
        P_ps = psum_ep.tile([P, gpc], F32, tag="T3")
        nc.tensor.matmul(P_ps[:], lhsT=Wsb[:], rhs=ST_sb[:], start=True, stop=False)
        nc.tensor.matmul(P_ps[:], lhsT=brow[:], rhs=cnt_row[:], start=False, stop=True)
        P_sb = small.tile([P, gpc], F32)
        nc.vector.tensor_copy(P_sb[:], P_ps[:])
        nc.sync.dma_start(pooledT_ap[:, :], P_sb[:])

    nc.compile()
    return nc


# --------------------------------------------------------------------------
# entry point
# --------------------------------------------------------------------------

_graph_cache: dict = {}


def _get_graph(cfg: Cfg, n_loc_pad: int, cap: int, slots_pad: int):
    key = (cfg, n_loc_pad, cap, slots_pad)
    if key not in _graph_cache:
        _graph_cache[key] = build_graph(cfg, n_loc_pad, cap, slots_pad)
    return _graph_cache[key]


def kernel(x, eps, W_pred, b_pred, edge_index, batch):
    from concourse import bass_utils

    cfg = REAL
    in_maps, meta = shard_inputs(cfg, x, eps, W_pred, b_pred, edge_index, batch)
    nc = _get_graph(cfg, meta["n_loc_pad"], meta["cap"], meta["slots_pad"])
    res = bass_utils.run_bass_kernel_spmd(
        nc, in_maps, core_ids=list(range(cfg.n_cores))
    )
    return unshard(cfg, res.results, meta)
